# revision 1
# baseline (speedup 1.0000x reference)
"""Memory-efficient multi-head attention on 8 Trainium2 NeuronCores.

Sharding: tensor-parallel over heads (4 head-groups) x data-parallel over
batch (2) = 8 cores. Core c handles head group g = c % 4 (heads 4g..4g+3,
feature slice 512) of batch b = c // 4. Each core computes its Q/K/V
projections from sliced weights, attention for its 4 heads, and a partial
out-projection y_c = ao_c @ Wo[:, gs].T; the host sums the 4 partials per
batch and adds the output bias.

Matmuls run in float32r (fp32 bits through the PE at bf16 rate for moving
dim >= 256, ~1e-4 relative accuracy).
"""

import sys

if "/opt/trn_rl_repo" not in sys.path:
    sys.path.insert(0, "/opt/trn_rl_repo")

from contextlib import ExitStack

import numpy as np

import concourse.bacc as bacc
import concourse.mybir as mybir
import concourse.tile as tile
from concourse.bass_utils import run_bass_kernel_spmd

B, S, D, H = 2, 2048, 2048, 16
HD = 128               # head dim
G = 4                  # head groups (tensor-parallel degree)
HPG = H // G           # heads per group = 4
FC = HPG * HD          # per-core feature slice = 512
KC = D // 128          # contraction chunks = 16
SB = 4                 # seq blocks in phase 1 (512 cols each)
QB = 4                 # q blocks in phase 2 (512 each)
ST = S // 128          # seq tiles = 16
SCALE = float(HD) ** -0.5

F32 = mybir.dt.float32
F32R = mybir.dt.float32r

PROFILE = False        # set by test.py to collect an NTFF trace
LAST = {}              # exec_time_ns etc. stashed here when PROFILE

_cache = {}


def _build(masked: bool):
    nc = bacc.Bacc("TRN2", target_bir_lowering=False)

    xT = nc.dram_tensor("xT", (D, S), F32R, kind="ExternalInput")
    wqT = nc.dram_tensor("wqT", (D, FC), F32R, kind="ExternalInput")
    wkT = nc.dram_tensor("wkT", (D, FC), F32R, kind="ExternalInput")
    wvT = nc.dram_tensor("wvT", (D, FC), F32R, kind="ExternalInput")
    woT = nc.dram_tensor("woT", (FC, D), F32R, kind="ExternalInput")
    bq2 = nc.dram_tensor("bq2", (128, HPG), F32, kind="ExternalInput")
    bk2 = nc.dram_tensor("bk2", (128, HPG), F32, kind="ExternalInput")
    bvb = nc.dram_tensor("bvb", (128, FC), F32, kind="ExternalInput")
    ones_k = nc.dram_tensor("ones_k", (128, 1), F32R, kind="ExternalInput")
    mT = None
    if masked:
        mT = nc.dram_tensor("mT", (S, S), F32, kind="ExternalInput")
    y = nc.dram_tensor("y", (S, D), F32, kind="ExternalOutput")

    # DRAM views tiled to 128 partitions: (c p) f -> p c f
    xT_v = xT[:].rearrange("(c p) s -> p c s", p=128)
    wqT_v = wqT[:].rearrange("(c p) f -> p c f", p=128)
    wkT_v = wkT[:].rearrange("(c p) f -> p c f", p=128)
    wvT_v = wvT[:].rearrange("(c p) f -> p c f", p=128)
    woT_v = woT[:].rearrange("(c p) f -> p c f", p=128)
    mT_v = mT[:].rearrange("(c p) q -> p c q", p=128) if masked else None

    with tile.TileContext(nc) as tc, ExitStack() as top:
        const = top.enter_context(tc.tile_pool(name="const", bufs=1))
        qkv = top.enter_context(tc.tile_pool(name="qkv", bufs=1))

        t_bq = const.tile([128, HPG], F32, tag="bq")
        t_bk = const.tile([128, HPG], F32, tag="bk")
        t_bvb = const.tile([128, FC], F32, tag="bvb")
        t_ok = const.tile([128, 1], F32R, tag="ok")
        nc.gpsimd.dma_start(t_bq[:], bq2[:])
        nc.gpsimd.dma_start(t_bk[:], bk2[:])
        nc.gpsimd.dma_start(t_bvb[:], bvb[:])
        nc.gpsimd.dma_start(t_ok[:], ones_k[:])

        QT = [qkv.tile([128, S], F32R, tag=f"qt{h}", name=f"qt{h}") for h in range(HPG)]
        KT = [qkv.tile([128, S], F32R, tag=f"kt{h}", name=f"kt{h}") for h in range(HPG)]
        V = [qkv.tile([128, FC], F32R, tag=f"v{st}", name=f"v{st}") for st in range(ST)]

        # ---- phase 1: projections ------------------------------------
        with tc.tile_pool(name="p1x", bufs=1) as p1x, \
             tc.tile_pool(name="p1w", bufs=1) as p1w, \
             tc.tile_pool(name="p1ps", bufs=1, space="PSUM") as p1ps:
            for sb in range(SB):
                ssl = slice(sb * 512, (sb + 1) * 512)
                xbt = []
                for xc in range(4):
                    t = p1x.tile([128, 4, 512], F32R, tag="xb", bufs=6,
                                 name=f"xb{xc}")
                    nc.scalar.dma_start(
                        t[:], xT_v[:, 4 * xc : 4 * xc + 4, ssl]
                    )
                    xbt.append(t)

                for wv_, bias_t, store in (
                    (wqT_v, t_bq, QT),
                    (wkT_v, t_bk, KT),
                ):
                    ps = [
                        p1ps.tile([128, 512], F32, tag=f"ps{mt}", bufs=2, name=f"ps{mt}")
                        for mt in range(HPG)
                    ]
                    for ck in range(4):
                        wt = p1w.tile([128, 4, FC], F32R, tag="w", bufs=5)
                        nc.sync.dma_start(
                            wt[:], wv_[:, 4 * ck : 4 * ck + 4, :]
                        )
                        for kk in range(4):
                            kc = 4 * ck + kk
                            for mt in range(HPG):
                                nc.tensor.matmul(
                                    ps[mt][:],
                                    wt[:, kk, mt * 128 : (mt + 1) * 128],
                                    xbt[kc // 4][:, kc % 4, :],
                                    start=(kc == 0),
                                    stop=(kc == KC - 1),
                                )
                    for mt in range(HPG):
                        nc.scalar.activation(
                            store[mt][:, ssl],
                            ps[mt][:],
                            mybir.ActivationFunctionType.Identity,
                            bias=bias_t[:, mt : mt + 1],
                            scale=1.0,
                        )

                # V projection (natural [seq, feat] layout)
                psv = [
                    p1ps.tile([128, 512], F32, tag=f"ps{j}", bufs=2, name=f"psv{j}")
                    for j in range(4)
                ]
                for ck in range(4):
                    wt = p1w.tile([128, 4, FC], F32R, tag="w", bufs=5)
                    nc.sync.dma_start(wt[:], wvT_v[:, 4 * ck : 4 * ck + 4, :])
                    for kk in range(4):
                        kc = 4 * ck + kk
                        for j in range(4):
                            nc.tensor.matmul(
                                psv[j][:],
                                xbt[kc // 4][:, kc % 4, j * 128 : (j + 1) * 128],
                                wt[:, kk, :],
                                start=(kc == 0),
                                stop=(kc == KC - 1),
                            )
                for j in range(4):
                    nc.vector.tensor_add(
                        V[sb * 4 + j][:], psv[j][:], t_bvb[:]
                    )

        # ---- phase 2: attention --------------------------------------
        with tc.tile_pool(name="ao", bufs=1) as aop, \
             tc.tile_pool(name="p3w", bufs=1) as p3w:
            aoT = [aop.tile([128, S], F32R, tag=f"ao{h}", name=f"ao{h}") for h in range(HPG)]
            WoS = [p3w.tile([128, D], F32R, tag=f"wo{fc}", name=f"wo{fc}") for fc in range(HPG)]

            with tc.tile_pool(name="p2s", bufs=1) as p2s, \
                 tc.tile_pool(name="p2ps", bufs=1, space="PSUM") as p2ps:
                _attention(nc, tc, p2s, p2ps, masked, mT_v, QT, KT, V, aoT,
                           t_ok)
            for fc in range(HPG):
                nc.sync.dma_start(WoS[fc][:], woT_v[:, fc, :])

            # ---- phase 3: out-projection -----------------------------
            with tc.tile_pool(name="p3s", bufs=1) as p3s, \
                 tc.tile_pool(name="p3ps", bufs=1, space="PSUM") as p3ps:
                for st in range(ST):
                    stsl = slice(st * 128, (st + 1) * 128)
                    psy = p3ps.tile([128, D], F32, tag="psy", bufs=2)
                    for fc in range(HPG):
                        for dcb in range(4):
                            nc.tensor.matmul(
                                psy[:, dcb * 512 : (dcb + 1) * 512],
                                aoT[fc][:, stsl],
                                WoS[fc][:, dcb * 512 : (dcb + 1) * 512],
                                start=(fc == 0),
                                stop=(fc == HPG - 1),
                            )
                    yt = p3s.tile([128, D], F32, tag="yt", bufs=3)
                    nc.scalar.copy(yt[:, 0:1024], psy[:, 0:1024])
                    nc.vector.tensor_copy(yt[:, 1024:2048], psy[:, 1024:2048])
                    nc.sync.dma_start(y[stsl, 0:1024], yt[:, 0:1024])
                    nc.sync.dma_start(y[stsl, 1024:2048], yt[:, 1024:2048])

    nc.finalize()
    return nc


def _attention(nc, tc, p2s, p2ps, masked, mT_v, QT, KT, V, aoT, t_ok):
    def scores(h, qsl, ktp):
        t = p2ps.tile([128, 1024], F32, tag="s", bufs=2, name="ps_s")
        for half in range(2):
            kt = 2 * ktp + half
            nc.tensor.matmul(
                t[:, half * 512 : (half + 1) * 512],
                KT[h][:, kt * 128 : (kt + 1) * 128],
                QT[h][:, qsl],
                start=True,
                stop=True,
            )
        return t

    for h in range(HPG):
        for qb in range(QB):
            qsl = slice(qb * 512, (qb + 1) * 512)
            ps_av = p2ps.tile([128, 512], F32, tag="av", bufs=2)
            ps_sum = p2ps.tile([1, 512], F32, tag="sum", bufs=2)
            cur = scores(h, qsl, 0)
            for ktp in range(8):
                nxt = scores(h, qsl, ktp + 1) if ktp < 7 else None
                et = p2s.tile([128, 1024], F32R, tag="expT", bufs=5)
                nc.scalar.activation(
                    et[:],
                    cur[:],
                    mybir.ActivationFunctionType.Exp,
                    scale=SCALE,
                )
                if masked:
                    mtile = p2s.tile(
                        [128, 2, 512], F32, tag="mtile", bufs=3
                    )
                    nc.sync.dma_start(
                        mtile[:],
                        mT_v[:, 2 * ktp : 2 * ktp + 2, qsl],
                    )
                    nc.vector.tensor_mul(
                        et[:],
                        et[:],
                        mtile[:].rearrange("p c q -> p (c q)"),
                    )
                for half in range(2):
                    kt = 2 * ktp + half
                    esl = slice(half * 512, (half + 1) * 512)
                    nc.tensor.matmul(
                        ps_av[:],
                        V[kt][:, h * 128 : (h + 1) * 128],
                        et[:, esl],
                        start=(kt == 0),
                        stop=(kt == ST - 1),
                    )
                    nc.tensor.matmul(
                        ps_sum[:],
                        t_ok[:],
                        et[:, esl],
                        start=(kt == 0),
                        stop=(kt == ST - 1),
                    )
                cur = nxt
            rs = p2s.tile([1, 512], F32, tag="rs", bufs=2, name="rs")
            nc.vector.reciprocal(rs[:], ps_sum[:])
            bc = p2s.tile([128, 512], F32, tag="bc", bufs=2, name="bc")
            nc.gpsimd.partition_broadcast(bc[:], rs[:])
            nc.vector.tensor_mul(aoT[h][:, qsl], ps_av[:], bc[:])


def kernel(x, mask, Wq, bq, Wk, bk, Wv, bv, Wo, bo):
    x = np.asarray(x, dtype=np.float32)
    mask = np.asarray(mask)
    Wq, bq = np.asarray(Wq, np.float32), np.asarray(bq, np.float32)
    Wk, bk = np.asarray(Wk, np.float32), np.asarray(bk, np.float32)
    Wv, bv = np.asarray(Wv, np.float32), np.asarray(bv, np.float32)
    Wo, bo = np.asarray(Wo, np.float32), np.asarray(bo, np.float32)

    masked = bool((mask == 0).any())
    key = masked
    if key not in _cache:
        _cache[key] = _build(masked)
    nc = _cache[key]

    xTb = [np.ascontiguousarray(x[b].T) for b in range(B)]
    mTb = None
    if masked:
        mTb = [
            np.ascontiguousarray((mask[b, 0] != 0).T.astype(np.float32))
            for b in range(B)
        ]
    ok = np.ones((128, 1), np.float32)

    in_maps = []
    for c in range(8):
        g, b = c % G, c // G
        gs = slice(g * FC, (g + 1) * FC)
        m = {
            "xT": xTb[b],
            "wqT": np.ascontiguousarray(Wq[gs].T),
            "wkT": np.ascontiguousarray(Wk[gs].T),
            "wvT": np.ascontiguousarray(Wv[gs].T),
            "woT": np.ascontiguousarray(Wo[:, gs].T),
            "bq2": np.ascontiguousarray(bq[gs].reshape(HPG, 128).T),
            "bk2": np.ascontiguousarray(bk[gs].reshape(HPG, 128).T),
            "bvb": np.tile(bv[gs][None, :], (128, 1)),
            "ones_k": ok,
        }
        if masked:
            m["mT"] = mTb[b]
        in_maps.append(m)

    res = run_bass_kernel_spmd(
        nc, in_maps, core_ids=list(range(8)), trace=PROFILE
    )
    if PROFILE:
        LAST["exec_time_ns"] = res.exec_time_ns
        LAST["profile_json"] = res.profile_json
        LAST["trace"] = res.instructions_and_trace

    out = np.empty((B, S, D), np.float32)
    for b in range(B):
        acc = res.results[4 * b]["y"].astype(np.float64)
        for g in range(1, G):
            acc += res.results[4 * b + g]["y"]
        out[b] = (acc + bo).astype(np.float32)
    return out



# revision 4
# speedup vs baseline: 1.0503x; 1.0503x over previous
"""Memory-efficient multi-head attention on 8 Trainium2 NeuronCores.

Sharding: tensor-parallel over heads (4 head-groups) x data-parallel over
batch (2) = 8 cores. Core c handles head group g = c % 4 (heads 4g..4g+3,
feature slice 512) of batch b = c // 4. Each core computes its Q/K/V
projections from sliced weights, attention for its 4 heads, and a partial
out-projection y_c = ao_c @ Wo[:, gs].T; the host sums the 4 partials per
batch and adds the output bias.

All matmuls run in fp16 (same PE rate as bf16, ~3e-4 end-to-end error,
half the DMA/SBUF of fp32, and fast-weight-load eligible so LDWEIGHTS
hides behind the matmul stream — fp32r weights load serially at 4B and
cost ~100us across the kernel). fp8 was measured and rejected: attention
output is itself an average over ~2k keys, so elementwise quantization
noise in V or exp(scores) does NOT average away (2.7% each), and the
fp8 Q/K path measures 2.3e-2 against the 2e-2 tolerance.

Softmax: scores/exp per 1024-wide PSUM tile; row-sums via ones-vector
matmuls accumulated alongside attn@V; denominator broadcast across
partitions with a K=1 matmul, inverted with reciprocal_approx_fast
(exact reciprocal on [1,512] costs 3.3us vs ~0.7us here), and a single
VectorE multiply writes the normalized attention output.
"""

import sys

if "/opt/trn_rl_repo" not in sys.path:
    sys.path.insert(0, "/opt/trn_rl_repo")

from contextlib import ExitStack

import numpy as np

import concourse.bacc as bacc
import concourse.mybir as mybir
import concourse.tile as tile
from concourse.bass_utils import run_bass_kernel_spmd

B, S, D, H = 2, 2048, 2048, 16
HD = 128               # head dim
G = 4                  # head groups (tensor-parallel degree)
HPG = H // G           # heads per group = 4
FC = HPG * HD          # per-core feature slice = 512
KC = D // 128          # contraction chunks = 16
SB = 4                 # seq blocks (512 wide)
QB = 4                 # q blocks (512 wide)
ST = S // 128          # seq tiles = 16
SCALE = float(HD) ** -0.5

F32 = mybir.dt.float32
F16 = mybir.dt.float16

PROFILE = False        # set by test.py to collect an NTFF trace
LAST = {}              # exec_time_ns etc. stashed here when PROFILE

_cache = {}


def _build(masked: bool):
    nc = bacc.Bacc("TRN2", target_bir_lowering=False)

    xb = nc.dram_tensor("xb", (D, S), F16, kind="ExternalInput")
    wqh = nc.dram_tensor("wqh", (D, FC), F16, kind="ExternalInput")
    wkh = nc.dram_tensor("wkh", (D, FC), F16, kind="ExternalInput")
    wvh = nc.dram_tensor("wvh", (D, FC), F16, kind="ExternalInput")
    woh = nc.dram_tensor("woh", (FC, D), F16, kind="ExternalInput")
    bq2 = nc.dram_tensor("bq2", (128, HPG), F32, kind="ExternalInput")
    bk2 = nc.dram_tensor("bk2", (128, HPG), F32, kind="ExternalInput")
    bvb = nc.dram_tensor("bvb", (128, FC), F32, kind="ExternalInput")
    onesc = nc.dram_tensor("onesc", (128, 16), F16, kind="ExternalInput")
    onesr = nc.dram_tensor("onesr", (1, 128), F16, kind="ExternalInput")
    mT = None
    if masked:
        mT = nc.dram_tensor("mT", (S, S), F32, kind="ExternalInput")
    y = nc.dram_tensor("y", (S, D), F16, kind="ExternalOutput")

    xb_v = xb[:].rearrange("(c p) s -> p c s", p=128)
    wq_v = wqh[:].rearrange("(c p) f -> p c f", p=128)
    wk_v = wkh[:].rearrange("(c p) f -> p c f", p=128)
    wv_v = wvh[:].rearrange("(c p) f -> p c f", p=128)
    wo_v = woh[:].rearrange("(c p) f -> p c f", p=128)
    mT_v = mT[:].rearrange("(c p) q -> p c q", p=128) if masked else None

    EXP = mybir.ActivationFunctionType.Exp
    IDN = mybir.ActivationFunctionType.Identity

    with tile.TileContext(nc) as tc, ExitStack() as top:
        const = top.enter_context(tc.tile_pool(name="const", bufs=1))
        store = top.enter_context(tc.tile_pool(name="store", bufs=1))
        attp = top.enter_context(tc.tile_pool(name="attp", bufs=1))

        t_bq = const.tile([128, HPG], F32, tag="bq")
        t_bk = const.tile([128, HPG], F32, tag="bk")
        t_bvb = const.tile([128, FC], F32, tag="bvb")
        t_oc = const.tile([128, 16], F16, tag="oc")
        t_or = const.tile([1, 128], F16, tag="or")
        nc.gpsimd.dma_start(t_bq[:], bq2[:])
        nc.gpsimd.dma_start(t_bk[:], bk2[:])
        nc.gpsimd.dma_start(t_bvb[:], bvb[:])
        nc.gpsimd.dma_start(t_oc[:], onesc[:])
        nc.gpsimd.dma_start(t_or[:], onesr[:])

        QT = [store.tile([128, S], F16, tag=f"qt{h}", name=f"qt{h}")
              for h in range(HPG)]
        KT = [store.tile([128, S], F16, tag=f"kt{h}", name=f"kt{h}")
              for h in range(HPG)]
        V = [store.tile([128, FC], F16, tag=f"v{kt}", name=f"v{kt}")
             for kt in range(ST)]
        AO = [store.tile([128, S], F16, tag=f"ao{h}", name=f"ao{h}")
              for h in range(HPG)]
        WoS = [store.tile([128, D], F16, tag=f"wo{fc}", name=f"wo{fc}")
               for fc in range(HPG)]

        with tc.tile_pool(name="ps", bufs=1, space="PSUM") as psp:
            # ---- phase 1: Q/K/V projections --------------------------
            with tc.tile_pool(name="wp", bufs=1) as wp, \
                 tc.tile_pool(name="xp", bufs=1) as xp:
                t_wq = wp.tile([128, KC, FC], F16, tag="wq")
                t_wk = wp.tile([128, KC, FC], F16, tag="wk")
                t_wv = wp.tile([128, KC, FC], F16, tag="wv")
                nc.sync.dma_start(t_wq[:], wq_v[:])
                nc.sync.dma_start(t_wk[:], wk_v[:])
                nc.sync.dma_start(t_wv[:], wv_v[:])
                for fc in range(HPG):
                    nc.sync.dma_start(WoS[fc][:], wo_v[:, fc, :])

                for sb in range(SB):
                    ssl = slice(sb * 512, (sb + 1) * 512)
                    txb = xp.tile([128, KC, 512], F16, tag="xb", bufs=2)
                    nc.scalar.dma_start(txb[:], xb_v[:, :, ssl])
                    for wt, bias_t, dst in ((t_wq, t_bq, QT),
                                            (t_wk, t_bk, KT)):
                        for mt in range(HPG):
                            ps = psp.tile([128, 512], F32, tag="s", bufs=2,
                                          name=f"psqk{mt}")
                            for kc in range(KC):
                                nc.tensor.matmul(
                                    ps[:],
                                    wt[:, kc, mt * 128 : (mt + 1) * 128],
                                    txb[:, kc, :],
                                    start=(kc == 0),
                                    stop=(kc == KC - 1),
                                )
                            nc.scalar.activation(
                                dst[mt][:, ssl], ps[:], IDN,
                                bias=bias_t[:, mt : mt + 1], scale=1.0,
                            )
                    for j in range(4):
                        kt = sb * 4 + j
                        ps = psp.tile([128, 512], F32, tag="s", bufs=2,
                                      name=f"psv{j}")
                        for kc in range(KC):
                            nc.tensor.matmul(
                                ps[:],
                                txb[:, kc, j * 128 : (j + 1) * 128],
                                t_wv[:, kc, :],
                                start=(kc == 0),
                                stop=(kc == KC - 1),
                            )
                        nc.vector.tensor_add(V[kt][:], ps[:], t_bvb[:])

            # ---- phase 2: attention ----------------------------------
            for h in range(HPG):
                for qb in range(QB):
                    qsl = slice(qb * 512, (qb + 1) * 512)
                    ps_av = psp.tile([128, 512], F32, tag="av", bufs=2,
                                     name="av")
                    ps_sm = psp.tile([1, 512], F32, tag="sum", bufs=2,
                                     name="sm")

                    def scores(ktp):
                        t = psp.tile([128, 1024], F32, tag="s", bufs=2,
                                     name="ps_s")
                        for half in range(2):
                            kt = 2 * ktp + half
                            nc.tensor.matmul(
                                t[:, half * 512 : (half + 1) * 512],
                                KT[h][:, kt * 128 : (kt + 1) * 128],
                                QT[h][:, qsl],
                                start=True,
                                stop=True,
                            )
                        return t

                    cur = scores(0)
                    for ktp in range(8):
                        nxt = scores(ktp + 1) if ktp < 7 else None
                        et = attp.tile([128, 1024], F16, tag="et", bufs=4)
                        nc.scalar.activation(et[:], cur[:], EXP, scale=SCALE)
                        if masked:
                            mtile = attp.tile([128, 2, 512], F32,
                                              tag="mtile", bufs=3)
                            nc.sync.dma_start(
                                mtile[:],
                                mT_v[:, 2 * ktp : 2 * ktp + 2, qsl],
                            )
                            nc.vector.tensor_mul(
                                et[:], et[:],
                                mtile[:].rearrange("p c q -> p (c q)"),
                            )
                        for half in range(2):
                            kt = 2 * ktp + half
                            esl = slice(half * 512, (half + 1) * 512)
                            nc.tensor.matmul(
                                ps_av[:],
                                V[kt][:, h * 128 : (h + 1) * 128],
                                et[:, esl],
                                start=(kt == 0),
                                stop=(kt == ST - 1),
                            )
                            nc.tensor.matmul(
                                ps_sm[:],
                                t_oc[:, 0:1],
                                et[:, esl],
                                start=(kt == 0),
                                stop=(kt == ST - 1),
                            )
                        cur = nxt

                    # normalize: broadcast 1/sum via K=1 matmul
                    smrow = attp.tile([1, 512], F16, tag="smrow", bufs=2)
                    nc.vector.tensor_copy(smrow[:], ps_sm[:])
                    bc = psp.tile([128, 512], F32, tag="sum", bufs=2,
                                  name="bc")
                    nc.tensor.matmul(bc[:], t_or[:], smrow[:],
                                     start=True, stop=True)
                    bcr = attp.tile([128, 512], F32, tag="bcr", bufs=2)
                    nc.vector.reciprocal_approx_fast(bcr[:], bc[:])
                    nc.vector.tensor_mul(AO[h][:, qsl], bcr[:], ps_av[:])

        # ---- phase 3: out-projection ------------------------------
        with tc.tile_pool(name="p3s", bufs=1) as p3s, \
             tc.tile_pool(name="p3ps", bufs=1, space="PSUM") as p3ps:
            for st in range(ST):
                stsl = slice(st * 128, (st + 1) * 128)
                psy = p3ps.tile([128, D], F32, tag="psy", bufs=2)
                for fc in range(HPG):
                    for dcb in range(4):
                        nc.tensor.matmul(
                            psy[:, dcb * 512 : (dcb + 1) * 512],
                            AO[fc][:, stsl],
                            WoS[fc][:, dcb * 512 : (dcb + 1) * 512],
                            start=(fc == 0),
                            stop=(fc == HPG - 1),
                        )
                yt = p3s.tile([128, D], F16, tag="yt", bufs=3)
                nc.scalar.copy(yt[:, 0:1024], psy[:, 0:1024])
                nc.vector.tensor_copy(yt[:, 1024:2048], psy[:, 1024:2048])
                nc.sync.dma_start(y[stsl, :], yt[:])

    nc.finalize()
    return nc


def _in_maps(x, mask, Wq, bq, Wk, bk, Wv, bv, Wo, bo, masked):
    oc = np.ones((128, 16), np.float16)
    orr = np.ones((1, 128), np.float16)
    per_batch = [
        np.ascontiguousarray(x[b].T).astype(np.float16) for b in range(B)
    ]
    mTb = None
    if masked:
        mTb = [
            np.ascontiguousarray((mask[b, 0] != 0).T.astype(np.float32))
            for b in range(B)
        ]
    in_maps = []
    for c in range(8):
        g, b = c % G, c // G
        gs = slice(g * FC, (g + 1) * FC)
        m = {
            "xb": per_batch[b],
            "wqh": np.ascontiguousarray(Wq[gs].T).astype(np.float16),
            "wkh": np.ascontiguousarray(Wk[gs].T).astype(np.float16),
            "wvh": np.ascontiguousarray(Wv[gs].T).astype(np.float16),
            "woh": np.ascontiguousarray(Wo[:, gs].T).astype(np.float16),
            "bq2": np.ascontiguousarray(bq[gs].reshape(HPG, 128).T),
            "bk2": np.ascontiguousarray(bk[gs].reshape(HPG, 128).T),
            "bvb": np.tile(bv[gs][None, :], (128, 1)).astype(np.float32),
            "onesc": oc,
            "onesr": orr,
        }
        if masked:
            m["mT"] = mTb[b]
        in_maps.append(m)
    return in_maps


def kernel(x, mask, Wq, bq, Wk, bk, Wv, bv, Wo, bo):
    x = np.asarray(x, dtype=np.float32)
    mask = np.asarray(mask)
    Wq, bq = np.asarray(Wq, np.float32), np.asarray(bq, np.float32)
    Wk, bk = np.asarray(Wk, np.float32), np.asarray(bk, np.float32)
    Wv, bv = np.asarray(Wv, np.float32), np.asarray(bv, np.float32)
    Wo, bo = np.asarray(Wo, np.float32), np.asarray(bo, np.float32)

    masked = bool((mask == 0).any())
    if masked not in _cache:
        _cache[masked] = _build(masked)
    nc = _cache[masked]

    in_maps = _in_maps(x, mask, Wq, bq, Wk, bk, Wv, bv, Wo, bo, masked)

    res = run_bass_kernel_spmd(
        nc, in_maps, core_ids=list(range(8)), trace=PROFILE
    )
    if PROFILE:
        LAST["exec_time_ns"] = res.exec_time_ns
        LAST["profile_json"] = res.profile_json
        LAST["trace"] = res.instructions_and_trace

    out = np.empty((B, S, D), np.float32)
    for b in range(B):
        acc = res.results[4 * b]["y"].astype(np.float64)
        for g in range(1, G):
            acc += res.results[4 * b + g]["y"].astype(np.float64)
        out[b] = (acc + bo).astype(np.float32)
    return out


# revision 11
# speedup vs baseline: 1.1714x; 1.1152x over previous
"""Memory-efficient multi-head attention on 8 Trainium2 NeuronCores.

Sharding: tensor-parallel over heads (4 head-groups) x data-parallel over
batch (2) = 8 cores. Core c handles head group g = c % 4 (heads 4g..4g+3,
feature slice 512) of batch b = c // 4. Each core computes its Q/K/V
projections from sliced weights, attention for its 4 heads, and a partial
out-projection y_c = ao_c @ Wo[:, gs].T; the host sums the 4 partials per
batch and adds the output bias.

All matmuls run in fp16 (same PE rate as bf16, ~3e-4 end-to-end error,
half the DMA/SBUF of fp32, and fast-weight-load eligible so LDWEIGHTS
hides behind the matmul stream — fp32r weights load serially at 4B and
cost ~100us across the kernel). fp8 was measured and rejected: attention
output is itself an average over ~2k keys, so elementwise quantization
noise in V or exp(scores) does NOT average away (2.7% each), and the
fp8 Q/K path measures 2.3e-2 against the 2e-2 tolerance.

Softmax: scores/exp per 1024-wide PSUM tile; row-sums via ones-vector
matmuls accumulated alongside attn@V; denominator broadcast across
partitions with a K=1 matmul, inverted with reciprocal_approx_fast
(exact reciprocal on [1,512] costs 3.3us vs ~0.7us here), and a single
VectorE multiply writes the normalized attention output.
"""

import sys

if "/opt/trn_rl_repo" not in sys.path:
    sys.path.insert(0, "/opt/trn_rl_repo")

from contextlib import ExitStack

import numpy as np

import concourse.bacc as bacc
import concourse.mybir as mybir
import concourse.tile as tile
from concourse.bass_utils import run_bass_kernel_spmd

B, S, D, H = 2, 2048, 2048, 16
HD = 128               # head dim
G = 4                  # head groups (tensor-parallel degree)
HPG = H // G           # heads per group = 4
FC = HPG * HD          # per-core feature slice = 512
KC = D // 128          # contraction chunks = 16
SB = 4                 # seq blocks (512 wide)
QB = 4                 # q blocks (512 wide)
ST = S // 128          # seq tiles = 16
SCALE = float(HD) ** -0.5

F32 = mybir.dt.float32
F16 = mybir.dt.float16

PROFILE = False        # set by test.py to collect an NTFF trace
LAST = {}              # exec_time_ns etc. stashed here when PROFILE

_cache = {}


def _build(masked: bool):
    nc = bacc.Bacc("TRN2", target_bir_lowering=False)

    xb = nc.dram_tensor("xb", (D, S), F16, kind="ExternalInput")
    wqh = nc.dram_tensor("wqh", (D, FC), F16, kind="ExternalInput")
    wkh = nc.dram_tensor("wkh", (D, FC), F16, kind="ExternalInput")
    wvh = nc.dram_tensor("wvh", (D, FC), F16, kind="ExternalInput")
    woh = nc.dram_tensor("woh", (FC, D), F16, kind="ExternalInput")
    bq2 = nc.dram_tensor("bq2", (128, HPG), F32, kind="ExternalInput")
    bk2 = nc.dram_tensor("bk2", (128, HPG), F32, kind="ExternalInput")
    bvb = nc.dram_tensor("bvb", (128, FC), F32, kind="ExternalInput")
    onesm = nc.dram_tensor("onesm", (128, 128), F16, kind="ExternalInput")
    mT = None
    if masked:
        mT = nc.dram_tensor("mT", (S, S), F32, kind="ExternalInput")
    y = nc.dram_tensor("y", (S, D), F16, kind="ExternalOutput")

    xb_v = xb[:].rearrange("(c p) s -> p c s", p=128)
    wq_v = wqh[:].rearrange("(c p) f -> p c f", p=128)
    wk_v = wkh[:].rearrange("(c p) f -> p c f", p=128)
    wv_v = wvh[:].rearrange("(c p) f -> p c f", p=128)
    wo_v = woh[:].rearrange("(c p) f -> p c f", p=128)
    mT_v = mT[:].rearrange("(c p) q -> p c q", p=128) if masked else None

    EXP = mybir.ActivationFunctionType.Exp
    IDN = mybir.ActivationFunctionType.Identity

    with tile.TileContext(nc) as tc, ExitStack() as top:
        const = top.enter_context(tc.tile_pool(name="const", bufs=1))
        store = top.enter_context(tc.tile_pool(name="store", bufs=1))
        attp = top.enter_context(tc.tile_pool(name="attp", bufs=1))

        t_bq = const.tile([128, HPG], F32, tag="bq")
        t_bk = const.tile([128, HPG], F32, tag="bk")
        t_bvb = const.tile([128, FC], F32, tag="bvb")
        t_ones = const.tile([128, 128], F16, tag="ones")
        nc.gpsimd.dma_start(t_bq[:], bq2[:])
        nc.gpsimd.dma_start(t_bk[:], bk2[:])
        nc.gpsimd.dma_start(t_bvb[:], bvb[:])
        nc.gpsimd.dma_start(t_ones[:], onesm[:])

        QT = [store.tile([128, S], F16, tag=f"qt{h}", name=f"qt{h}")
              for h in range(HPG)]
        KT = [store.tile([128, S], F16, tag=f"kt{h}", name=f"kt{h}")
              for h in range(HPG)]
        V = [store.tile([128, FC], F16, tag=f"v{kt}", name=f"v{kt}")
             for kt in range(ST)]
        AO = [store.tile([128, S], F16, tag=f"ao{h}", name=f"ao{h}")
              for h in range(HPG)]
        WoS = [store.tile([128, D], F16, tag=f"wo{fc}", name=f"wo{fc}")
               for fc in range(HPG)]

        with tc.tile_pool(name="ps", bufs=1, space="PSUM") as psp:
            # ---- phase 1: Q/K/V projections --------------------------
            with tc.tile_pool(name="wp", bufs=1) as wp, \
                 tc.tile_pool(name="xp", bufs=1) as xp:
                t_wq = wp.tile([128, KC, FC], F16, tag="wq")
                t_wk = wp.tile([128, KC, FC], F16, tag="wk")
                t_wv = wp.tile([128, KC, FC], F16, tag="wv")
                # chunked so the first projection matmuls start as soon as
                # the first contraction chunks land
                for c4 in range(4):
                    csl = slice(4 * c4, 4 * c4 + 4)
                    nc.sync.dma_start(t_wq[:, csl, :], wq_v[:, csl, :])
                nc.sync.dma_start(t_wk[:], wk_v[:])
                nc.sync.dma_start(t_wv[:], wv_v[:])
                for fc in range(HPG):
                    nc.sync.dma_start(WoS[fc][:], wo_v[:, fc, :])

                for sb in range(SB):
                    ssl = slice(sb * 512, (sb + 1) * 512)
                    txb = xp.tile([128, KC, 512], F16, tag="xb", bufs=2)
                    if sb == 0:
                        for c4 in range(4):
                            csl = slice(4 * c4, 4 * c4 + 4)
                            nc.scalar.dma_start(txb[:, csl, :],
                                                xb_v[:, csl, ssl])
                    else:
                        nc.scalar.dma_start(txb[:], xb_v[:, :, ssl])
                    for wt, bias_t, dst in ((t_wq, t_bq, QT),
                                            (t_wk, t_bk, KT)):
                        for mt in range(HPG):
                            ps = psp.tile([128, 512], F32, tag="s", bufs=2,
                                          name=f"psqk{mt}")
                            for kc in range(KC):
                                nc.tensor.matmul(
                                    ps[:],
                                    wt[:, kc, mt * 128 : (mt + 1) * 128],
                                    txb[:, kc, :],
                                    start=(kc == 0),
                                    stop=(kc == KC - 1),
                                )
                            nc.scalar.activation(
                                dst[mt][:, ssl], ps[:], IDN,
                                bias=bias_t[:, mt : mt + 1], scale=1.0,
                            )
                    for j in range(4):
                        kt = sb * 4 + j
                        ps = psp.tile([128, 512], F32, tag="s", bufs=2,
                                      name=f"psv{j}")
                        for kc in range(KC):
                            nc.tensor.matmul(
                                ps[:],
                                txb[:, kc, j * 128 : (j + 1) * 128],
                                t_wv[:, kc, :],
                                start=(kc == 0),
                                stop=(kc == KC - 1),
                            )
                        nc.vector.tensor_add(V[kt][:], ps[:], t_bvb[:])

            # ---- phase 2: attention ----------------------------------
            for h in range(HPG):
                for qb in range(QB):
                    qsl = slice(qb * 512, (qb + 1) * 512)
                    ps_av = psp.tile([128, 512], F32, tag="av", bufs=2,
                                     name="av")
                    # [128,128] ones stationary: every matmul in the kernel
                    # keeps the same tile shape (shape switches cost ~95ns
                    # of exposed LDWEIGHTS each), and the output arrives
                    # already broadcast across partitions.
                    ps_sm = psp.tile([128, 512], F32, tag="sum", bufs=2,
                                     name="sm")

                    def scores(ktp):
                        t = psp.tile([128, 1024], F32, tag="s", bufs=2,
                                     name="ps_s")
                        for half in range(2):
                            kt = 2 * ktp + half
                            nc.tensor.matmul(
                                t[:, half * 512 : (half + 1) * 512],
                                KT[h][:, kt * 128 : (kt + 1) * 128],
                                QT[h][:, qsl],
                                start=True,
                                stop=True,
                            )
                        return t

                    cur = scores(0)
                    for ktp in range(8):
                        nxt = scores(ktp + 1) if ktp < 7 else None
                        et = attp.tile([128, 1024], F16, tag="et", bufs=4)
                        nc.scalar.activation(et[:], cur[:], EXP, scale=SCALE)
                        if masked:
                            mtile = attp.tile([128, 2, 512], F32,
                                              tag="mtile", bufs=3)
                            nc.sync.dma_start(
                                mtile[:],
                                mT_v[:, 2 * ktp : 2 * ktp + 2, qsl],
                            )
                            nc.vector.tensor_mul(
                                et[:], et[:],
                                mtile[:].rearrange("p c q -> p (c q)"),
                            )
                        for half in range(2):
                            kt = 2 * ktp + half
                            esl = slice(half * 512, (half + 1) * 512)
                            nc.tensor.matmul(
                                ps_av[:],
                                V[kt][:, h * 128 : (h + 1) * 128],
                                et[:, esl],
                                start=(kt == 0),
                                stop=(kt == ST - 1),
                            )
                            nc.tensor.matmul(
                                ps_sm[:],
                                t_ones[:],
                                et[:, esl],
                                start=(kt == 0),
                                stop=(kt == ST - 1),
                            )
                        cur = nxt

                    # normalize: ps_sm rows are already the broadcast sums
                    bcr = attp.tile([128, 512], F32, tag="bcr", bufs=2)
                    nc.vector.reciprocal_approx_fast(bcr[:], ps_sm[:])
                    nc.vector.tensor_mul(AO[h][:, qsl], bcr[:], ps_av[:])

        # ---- phase 3: out-projection ------------------------------
        with tc.tile_pool(name="p3s", bufs=1) as p3s, \
             tc.tile_pool(name="p3ps", bufs=1, space="PSUM") as p3ps:
            for st in range(ST):
                stsl = slice(st * 128, (st + 1) * 128)
                psy = p3ps.tile([128, D], F32, tag="psy", bufs=2)
                for fc in range(HPG):
                    for dcb in range(4):
                        nc.tensor.matmul(
                            psy[:, dcb * 512 : (dcb + 1) * 512],
                            AO[fc][:, stsl],
                            WoS[fc][:, dcb * 512 : (dcb + 1) * 512],
                            start=(fc == 0),
                            stop=(fc == HPG - 1),
                        )
                yt = p3s.tile([128, D], F16, tag="yt", bufs=3)
                nc.scalar.copy(yt[:, 0:1024], psy[:, 0:1024])
                nc.vector.tensor_copy(yt[:, 1024:2048], psy[:, 1024:2048])
                nc.sync.dma_start(y[stsl, :], yt[:])

    nc.finalize()
    return nc


def _in_maps(x, mask, Wq, bq, Wk, bk, Wv, bv, Wo, bo, masked):
    om = np.ones((128, 128), np.float16)
    per_batch = [
        np.ascontiguousarray(x[b].T).astype(np.float16) for b in range(B)
    ]
    mTb = None
    if masked:
        mTb = [
            np.ascontiguousarray((mask[b, 0] != 0).T.astype(np.float32))
            for b in range(B)
        ]
    in_maps = []
    for c in range(8):
        g, b = c % G, c // G
        gs = slice(g * FC, (g + 1) * FC)
        m = {
            "xb": per_batch[b],
            "wqh": np.ascontiguousarray(Wq[gs].T).astype(np.float16),
            "wkh": np.ascontiguousarray(Wk[gs].T).astype(np.float16),
            "wvh": np.ascontiguousarray(Wv[gs].T).astype(np.float16),
            "woh": np.ascontiguousarray(Wo[:, gs].T).astype(np.float16),
            "bq2": np.ascontiguousarray(bq[gs].reshape(HPG, 128).T),
            "bk2": np.ascontiguousarray(bk[gs].reshape(HPG, 128).T),
            "bvb": np.tile(bv[gs][None, :], (128, 1)).astype(np.float32),
            "onesm": om,
        }
        if masked:
            m["mT"] = mTb[b]
        in_maps.append(m)
    return in_maps


def kernel(x, mask, Wq, bq, Wk, bk, Wv, bv, Wo, bo):
    x = np.asarray(x, dtype=np.float32)
    mask = np.asarray(mask)
    Wq, bq = np.asarray(Wq, np.float32), np.asarray(bq, np.float32)
    Wk, bk = np.asarray(Wk, np.float32), np.asarray(bk, np.float32)
    Wv, bv = np.asarray(Wv, np.float32), np.asarray(bv, np.float32)
    Wo, bo = np.asarray(Wo, np.float32), np.asarray(bo, np.float32)

    masked = bool((mask == 0).any())
    if masked not in _cache:
        _cache[masked] = _build(masked)
    nc = _cache[masked]

    in_maps = _in_maps(x, mask, Wq, bq, Wk, bk, Wv, bv, Wo, bo, masked)

    res = run_bass_kernel_spmd(
        nc, in_maps, core_ids=list(range(8)), trace=PROFILE
    )
    if PROFILE:
        LAST["exec_time_ns"] = res.exec_time_ns
        LAST["profile_json"] = res.profile_json
        LAST["trace"] = res.instructions_and_trace

    out = np.empty((B, S, D), np.float32)
    for b in range(B):
        acc = res.results[4 * b]["y"].astype(np.float64)
        for g in range(1, G):
            acc += res.results[4 * b + g]["y"].astype(np.float64)
        out[b] = (acc + bo).astype(np.float32)
    return out


# revision 14
# speedup vs baseline: 1.1836x; 1.0104x over previous
"""Memory-efficient multi-head attention on 8 Trainium2 NeuronCores.

Sharding: tensor-parallel over heads (4 head-groups) x data-parallel over
batch (2) = 8 cores. Core c handles head group g = c % 4 (heads 4g..4g+3,
feature slice 512) of batch b = c // 4. Each core computes its Q/K/V
projections from sliced weights, attention for its 4 heads, and a partial
out-projection y_c = ao_c @ Wo[:, gs].T; the host sums the 4 partials per
batch and adds the output bias.

All matmuls run in fp16 (same PE rate as bf16, ~3e-4 end-to-end error,
half the DMA/SBUF of fp32, and fast-weight-load eligible so LDWEIGHTS
hides behind the matmul stream — fp32r weights load serially at 4B and
cost ~100us across the kernel). fp8 was measured and rejected: attention
output is itself an average over ~2k keys, so elementwise quantization
noise in V or exp(scores) does NOT average away (2.7% each), and the
fp8 Q/K path measures 2.3e-2 against the 2e-2 tolerance.

Softmax: scores/exp per 1024-wide PSUM tile; row-sums via ones-vector
matmuls accumulated alongside attn@V; denominator broadcast across
partitions with a K=1 matmul, inverted with reciprocal_approx_fast
(exact reciprocal on [1,512] costs 3.3us vs ~0.7us here), and a single
VectorE multiply writes the normalized attention output.
"""

import sys

if "/opt/trn_rl_repo" not in sys.path:
    sys.path.insert(0, "/opt/trn_rl_repo")

from contextlib import ExitStack

import numpy as np

import concourse.bacc as bacc
import concourse.mybir as mybir
import concourse.tile as tile
from concourse.bass_utils import run_bass_kernel_spmd

B, S, D, H = 2, 2048, 2048, 16
HD = 128               # head dim
G = 4                  # head groups (tensor-parallel degree)
HPG = H // G           # heads per group = 4
FC = HPG * HD          # per-core feature slice = 512
KC = D // 128          # contraction chunks = 16
SB = 4                 # seq blocks (512 wide)
QB = 4                 # q blocks (512 wide)
ST = S // 128          # seq tiles = 16
SCALE = float(HD) ** -0.5

F32 = mybir.dt.float32
F16 = mybir.dt.float16

PROFILE = False        # set by test.py to collect an NTFF trace
LAST = {}              # exec_time_ns etc. stashed here when PROFILE

_cache = {}


def _build(masked: bool):
    nc = bacc.Bacc("TRN2", target_bir_lowering=False)

    xb = nc.dram_tensor("xb", (D, S), F16, kind="ExternalInput")
    wqh = nc.dram_tensor("wqh", (D, FC), F16, kind="ExternalInput")
    wkh = nc.dram_tensor("wkh", (D, FC), F16, kind="ExternalInput")
    wvh = nc.dram_tensor("wvh", (D, FC), F16, kind="ExternalInput")
    woh = nc.dram_tensor("woh", (FC, D), F16, kind="ExternalInput")
    bq2 = nc.dram_tensor("bq2", (128, HPG), F32, kind="ExternalInput")
    bk2 = nc.dram_tensor("bk2", (128, HPG), F32, kind="ExternalInput")
    bvb = nc.dram_tensor("bvb", (128, FC), F32, kind="ExternalInput")
    onesm = nc.dram_tensor("onesm", (128, 128), F16, kind="ExternalInput")
    mT = None
    if masked:
        mT = nc.dram_tensor("mT", (S, S), F32, kind="ExternalInput")
    y = nc.dram_tensor("y", (S, D), F16, kind="ExternalOutput")

    xb_v = xb[:].rearrange("(c p) s -> p c s", p=128)
    wq_v = wqh[:].rearrange("(c p) f -> p c f", p=128)
    wk_v = wkh[:].rearrange("(c p) f -> p c f", p=128)
    wv_v = wvh[:].rearrange("(c p) f -> p c f", p=128)
    wo_v = woh[:].rearrange("(c p) f -> p c f", p=128)
    mT_v = mT[:].rearrange("(c p) q -> p c q", p=128) if masked else None

    EXP = mybir.ActivationFunctionType.Exp
    IDN = mybir.ActivationFunctionType.Identity

    with tile.TileContext(nc) as tc, ExitStack() as top:
        const = top.enter_context(tc.tile_pool(name="const", bufs=1))
        store = top.enter_context(tc.tile_pool(name="store", bufs=1))
        attp = top.enter_context(tc.tile_pool(name="attp", bufs=1))

        t_bq = const.tile([128, HPG], F32, tag="bq")
        t_bk = const.tile([128, HPG], F32, tag="bk")
        t_bvb = const.tile([128, FC], F32, tag="bvb")
        t_ones = const.tile([128, 128], F16, tag="ones")
        nc.gpsimd.dma_start(t_bq[:], bq2[:])
        nc.gpsimd.dma_start(t_bk[:], bk2[:])
        nc.gpsimd.dma_start(t_bvb[:], bvb[:])
        nc.gpsimd.dma_start(t_ones[:], onesm[:])

        QT = [store.tile([128, S], F16, tag=f"qt{h}", name=f"qt{h}")
              for h in range(HPG)]
        KT = [store.tile([128, S], F16, tag=f"kt{h}", name=f"kt{h}")
              for h in range(HPG)]
        V = [store.tile([128, FC], F16, tag=f"v{kt}", name=f"v{kt}")
             for kt in range(ST)]
        AO = [store.tile([128, S], F16, tag=f"ao{h}", name=f"ao{h}")
              for h in range(HPG)]
        WoS = [store.tile([128, D], F16, tag=f"wo{fc}", name=f"wo{fc}")
               for fc in range(HPG)]

        with tc.tile_pool(name="ps", bufs=1, space="PSUM") as psp:
            # ---- phase 1: Q/K/V projections --------------------------
            with tc.tile_pool(name="wp", bufs=1) as wp, \
                 tc.tile_pool(name="xp", bufs=1) as xp:
                t_wq = wp.tile([128, KC, FC], F16, tag="wq")
                t_wk = wp.tile([128, KC, FC], F16, tag="wk")
                t_wv = wp.tile([128, KC, FC], F16, tag="wv")
                # chunked so the first projection matmuls start as soon as
                # the first contraction chunks land
                for c4 in range(4):
                    csl = slice(4 * c4, 4 * c4 + 4)
                    nc.sync.dma_start(t_wq[:, csl, :], wq_v[:, csl, :])
                nc.sync.dma_start(t_wk[:], wk_v[:])
                nc.sync.dma_start(t_wv[:], wv_v[:])
                for fc in range(HPG):
                    nc.sync.dma_start(WoS[fc][:], wo_v[:, fc, :])

                for sb in range(SB):
                    ssl = slice(sb * 512, (sb + 1) * 512)
                    txb = xp.tile([128, KC, 512], F16, tag="xb", bufs=2)
                    if sb == 0:
                        for c4 in range(4):
                            csl = slice(4 * c4, 4 * c4 + 4)
                            nc.scalar.dma_start(txb[:, csl, :],
                                                xb_v[:, csl, ssl])
                    else:
                        nc.scalar.dma_start(txb[:], xb_v[:, :, ssl])
                    # chunk-progressive sweep over 4 concurrent PSUM groups
                    # so matmul consumption tracks the weight/x DMA arrival
                    # instead of each group demanding all 16 chunks at once
                    for wt, bias_t, dst in ((t_wq, t_bq, QT),
                                            (t_wk, t_bk, KT)):
                        pss = [psp.tile([128, 512], F32, tag="p1", bufs=4,
                                        name=f"psqk{mt}")
                               for mt in range(HPG)]
                        for c4 in range(4):
                            for mt in range(HPG):
                                for kc in range(4 * c4, 4 * c4 + 4):
                                    nc.tensor.matmul(
                                        pss[mt][:],
                                        wt[:, kc,
                                           mt * 128 : (mt + 1) * 128],
                                        txb[:, kc, :],
                                        start=(kc == 0),
                                        stop=(kc == KC - 1),
                                    )
                        for mt in range(HPG):
                            nc.scalar.activation(
                                dst[mt][:, ssl], pss[mt][:], IDN,
                                bias=bias_t[:, mt : mt + 1], scale=1.0,
                            )
                    psv = [psp.tile([128, 512], F32, tag="p1", bufs=4,
                                    name=f"psv{j}")
                           for j in range(4)]
                    for c4 in range(4):
                        for j in range(4):
                            for kc in range(4 * c4, 4 * c4 + 4):
                                nc.tensor.matmul(
                                    psv[j][:],
                                    txb[:, kc, j * 128 : (j + 1) * 128],
                                    t_wv[:, kc, :],
                                    start=(kc == 0),
                                    stop=(kc == KC - 1),
                                )
                    for j in range(4):
                        kt = sb * 4 + j
                        nc.vector.tensor_add(V[kt][:], psv[j][:], t_bvb[:])

            # ---- phase 2: attention ----------------------------------
            for h in range(HPG):
                for qb in range(QB):
                    qsl = slice(qb * 512, (qb + 1) * 512)
                    ps_av = psp.tile([128, 512], F32, tag="p1", bufs=4,
                                     name="av")
                    # [128,128] ones stationary: every matmul in the kernel
                    # keeps the same tile shape (shape switches cost ~95ns
                    # of exposed LDWEIGHTS each), and the output arrives
                    # already broadcast across partitions.
                    ps_sm = psp.tile([128, 512], F32, tag="p1", bufs=4,
                                     name="sm")

                    def scores(ktp):
                        t = psp.tile([128, 1024], F32, tag="s", bufs=2,
                                     name="ps_s")
                        for half in range(2):
                            kt = 2 * ktp + half
                            nc.tensor.matmul(
                                t[:, half * 512 : (half + 1) * 512],
                                KT[h][:, kt * 128 : (kt + 1) * 128],
                                QT[h][:, qsl],
                                start=True,
                                stop=True,
                            )
                        return t

                    cur = scores(0)
                    for ktp in range(8):
                        nxt = scores(ktp + 1) if ktp < 7 else None
                        et = attp.tile([128, 1024], F16, tag="et", bufs=4)
                        nc.scalar.activation(et[:], cur[:], EXP, scale=SCALE)
                        if masked:
                            mtile = attp.tile([128, 2, 512], F32,
                                              tag="mtile", bufs=3)
                            nc.sync.dma_start(
                                mtile[:],
                                mT_v[:, 2 * ktp : 2 * ktp + 2, qsl],
                            )
                            nc.vector.tensor_mul(
                                et[:], et[:],
                                mtile[:].rearrange("p c q -> p (c q)"),
                            )
                        for half in range(2):
                            kt = 2 * ktp + half
                            esl = slice(half * 512, (half + 1) * 512)
                            nc.tensor.matmul(
                                ps_av[:],
                                V[kt][:, h * 128 : (h + 1) * 128],
                                et[:, esl],
                                start=(kt == 0),
                                stop=(kt == ST - 1),
                            )
                            nc.tensor.matmul(
                                ps_sm[:],
                                t_ones[:],
                                et[:, esl],
                                start=(kt == 0),
                                stop=(kt == ST - 1),
                            )
                        cur = nxt

                    # normalize: ps_sm rows are already the broadcast sums
                    bcr = attp.tile([128, 512], F32, tag="bcr", bufs=2)
                    nc.vector.reciprocal_approx_fast(bcr[:], ps_sm[:])
                    nc.vector.tensor_mul(AO[h][:, qsl], bcr[:], ps_av[:])

        # ---- phase 3: out-projection ------------------------------
        with tc.tile_pool(name="p3s", bufs=1) as p3s, \
             tc.tile_pool(name="p3ps", bufs=1, space="PSUM") as p3ps:
            for st in range(ST):
                stsl = slice(st * 128, (st + 1) * 128)
                psy = p3ps.tile([128, D], F32, tag="psy", bufs=2)
                for fc in range(HPG):
                    for dcb in range(4):
                        nc.tensor.matmul(
                            psy[:, dcb * 512 : (dcb + 1) * 512],
                            AO[fc][:, stsl],
                            WoS[fc][:, dcb * 512 : (dcb + 1) * 512],
                            start=(fc == 0),
                            stop=(fc == HPG - 1),
                        )
                yt = p3s.tile([128, D], F16, tag="yt", bufs=3)
                nc.scalar.copy(yt[:, 0:1024], psy[:, 0:1024])
                nc.sync.dma_start(y[stsl, 0:1024], yt[:, 0:1024])
                nc.vector.tensor_copy(yt[:, 1024:2048], psy[:, 1024:2048])
                nc.sync.dma_start(y[stsl, 1024:2048], yt[:, 1024:2048])

    nc.finalize()
    return nc


def _in_maps(x, mask, Wq, bq, Wk, bk, Wv, bv, Wo, bo, masked):
    om = np.ones((128, 128), np.float16)
    per_batch = [
        np.ascontiguousarray(x[b].T).astype(np.float16) for b in range(B)
    ]
    mTb = None
    if masked:
        mTb = [
            np.ascontiguousarray((mask[b, 0] != 0).T.astype(np.float32))
            for b in range(B)
        ]
    in_maps = []
    for c in range(8):
        g, b = c % G, c // G
        gs = slice(g * FC, (g + 1) * FC)
        m = {
            "xb": per_batch[b],
            "wqh": np.ascontiguousarray(Wq[gs].T).astype(np.float16),
            "wkh": np.ascontiguousarray(Wk[gs].T).astype(np.float16),
            "wvh": np.ascontiguousarray(Wv[gs].T).astype(np.float16),
            "woh": np.ascontiguousarray(Wo[:, gs].T).astype(np.float16),
            "bq2": np.ascontiguousarray(bq[gs].reshape(HPG, 128).T),
            "bk2": np.ascontiguousarray(bk[gs].reshape(HPG, 128).T),
            "bvb": np.tile(bv[gs][None, :], (128, 1)).astype(np.float32),
            "onesm": om,
        }
        if masked:
            m["mT"] = mTb[b]
        in_maps.append(m)
    return in_maps


def kernel(x, mask, Wq, bq, Wk, bk, Wv, bv, Wo, bo):
    x = np.asarray(x, dtype=np.float32)
    mask = np.asarray(mask)
    Wq, bq = np.asarray(Wq, np.float32), np.asarray(bq, np.float32)
    Wk, bk = np.asarray(Wk, np.float32), np.asarray(bk, np.float32)
    Wv, bv = np.asarray(Wv, np.float32), np.asarray(bv, np.float32)
    Wo, bo = np.asarray(Wo, np.float32), np.asarray(bo, np.float32)

    masked = bool((mask == 0).any())
    if masked not in _cache:
        _cache[masked] = _build(masked)
    nc = _cache[masked]

    in_maps = _in_maps(x, mask, Wq, bq, Wk, bk, Wv, bv, Wo, bo, masked)

    res = run_bass_kernel_spmd(
        nc, in_maps, core_ids=list(range(8)), trace=PROFILE
    )
    if PROFILE:
        LAST["exec_time_ns"] = res.exec_time_ns
        LAST["profile_json"] = res.profile_json
        LAST["trace"] = res.instructions_and_trace

    out = np.empty((B, S, D), np.float32)
    for b in range(B):
        acc = res.results[4 * b]["y"].astype(np.float64)
        for g in range(1, G):
            acc += res.results[4 * b + g]["y"].astype(np.float64)
        out[b] = (acc + bo).astype(np.float32)
    return out
